# revision 3
# baseline (speedup 1.0000x reference)
"""Cross-attention kernel for 8 Trainium2 NeuronCores.

Sharding: core c => batch b = c//4, head-group g = c%4 (3 of 12 heads, 192 dims).
Each core projects q/k/v for its heads, does softmax attention, and computes a
partial output projection (row-split Wo); host sums the 4 partials per batch.

Key tricks:
  - mask compaction on host: only mask==1 key/value positions are shipped
    (~2048 of 4096), zero-padded to a multiple of 128. Padded rows have
    zeroed v and zeroed ones-column so they contribute 0 to both numerator
    and denominator => exact equivalence with the reference's -1e4 bias.
  - transposed layouts end to end (contraction dim on partitions), so no
    on-device transposes are needed.
  - softmax without max-subtraction (scores*scale ~ N(0,1), exp is safe in
    fp32) and without dividing the SxN score matrix: a ones-column appended
    to v yields the denominator Z per output row; only the 64xN attention
    output is normalized.
  - fp16 operands for all matmuls (fp32 PSUM accumulate).
"""

import numpy as np

import concourse.bass as bass
import concourse.mybir as mybir
import concourse.tile as tile
from concourse import bacc
from concourse.bass_utils import run_bass_kernel_spmd

H = 12
D = 768
HD = 64
SCALE = HD ** -0.5
NQ = 1024
HL = 3            # heads per core
HWID = HL * HD    # 192 head dims per core
DC = D // 128     # 6 contraction chunks

f16 = mybir.dt.float16
f32 = mybir.dt.float32

_programs = {}


def _build(SP: int):
    NCH = SP // 128
    nc = bacc.Bacc("TRN2", target_bir_lowering=False, debug=False, num_devices=8)

    qT = nc.dram_tensor("qT", [D, NQ], f16, kind="ExternalInput")
    kT = nc.dram_tensor("kT", [D, SP], f16, kind="ExternalInput")
    vT = nc.dram_tensor("vT", [D, SP], f16, kind="ExternalInput")
    mv = nc.dram_tensor("mv", [SP], f16, kind="ExternalInput")
    wqT = nc.dram_tensor("wqT", [D, HWID], f16, kind="ExternalInput")
    wkT = nc.dram_tensor("wkT", [D, HWID], f16, kind="ExternalInput")
    wvT = nc.dram_tensor("wvT", [D, HWID], f16, kind="ExternalInput")
    woT = nc.dram_tensor("woT", [HWID, D], f16, kind="ExternalInput")
    out = nc.dram_tensor("out", [NQ, D], f32, kind="ExternalOutput")

    EXPF = mybir.ActivationFunctionType.Exp

    with tile.TileContext(nc) as tc:
        with (
            tc.tile_pool(name="const", bufs=1) as cpool,
            tc.tile_pool(name="work", bufs=2) as wpool,
            tc.tile_pool(name="expp", bufs=3) as epool,
            tc.tile_pool(name="ps", bufs=3, space="PSUM") as pspool,
            tc.tile_pool(name="psa", bufs=1, space="PSUM") as psapool,
        ):
            # ---- input DMAs (transposed layouts: contraction dim on partitions)
            qT_in = cpool.tile([128, DC, NQ], f16)
            nc.sync.dma_start(qT_in[:], qT.ap().rearrange("(c p) n -> p c n", p=128))
            kT_in = cpool.tile([128, DC, SP], f16)
            nc.sync.dma_start(kT_in[:], kT.ap().rearrange("(c p) n -> p c n", p=128))
            vT_in = cpool.tile([128, DC, SP], f16)
            nc.sync.dma_start(vT_in[:], vT.ap().rearrange("(c p) n -> p c n", p=128))
            wq_in = cpool.tile([128, DC, HWID], f16)
            nc.sync.dma_start(wq_in[:], wqT.ap().rearrange("(c p) n -> p c n", p=128))
            wk_in = cpool.tile([128, DC, HWID], f16)
            nc.sync.dma_start(wk_in[:], wkT.ap().rearrange("(c p) n -> p c n", p=128))
            wv_in = cpool.tile([128, DC, HWID], f16)
            nc.sync.dma_start(wv_in[:], wvT.ap().rearrange("(c p) n -> p c n", p=128))
            wo_in = cpool.tile([128, 2, D], f16)
            nc.sync.dma_start(wo_in[:, 0, :], woT[0:128, :])
            nc.sync.dma_start(wo_in[0:64, 1, :], woT[128:HWID, :])
            msk = cpool.tile([128, NCH], f16)
            nc.sync.dma_start(msk[:], mv.ap().rearrange("(c p) -> p c", p=128))

            # ---- q / k projections -> q0/q1 (h0,h1 | h2), k0/k1, fp16
            q0 = cpool.tile([128, NQ], f16)
            q1 = cpool.tile([64, NQ], f16)
            k0 = cpool.tile([128, SP], f16)
            k1 = cpool.tile([64, SP], f16)
            for w_in, d0, d1, width in ((wq_in, q0, q1, NQ), (wk_in, k0, k1, SP)):
                for mt, (dst, mw) in enumerate(((d0, 128), (d1, 64))):
                    for sg in range(0, width, 1024):
                        sw = min(1024, width - sg)
                        ps = pspool.tile([mw, sw], f32, tag="ps")
                        for d in range(DC):
                            for nf in range(0, sw, 512):
                                wf = min(512, sw - nf)
                                nc.tensor.matmul(
                                    ps[:, nf:nf + wf],
                                    w_in[:, d, mt * 128:mt * 128 + mw],
                                    (qT_in if width == NQ else kT_in)[:, d, sg + nf:sg + nf + wf],
                                    start=(d == 0), stop=(d == DC - 1),
                                )
                        nc.vector.tensor_copy(dst[:, sg:sg + sw], ps[:])

            # ---- v projection into augmented layout [128, h, j, 65]
            vaug = cpool.tile([128, HL * NCH * 65], f16)
            vaug_r = vaug[:].rearrange("p (h j e) -> p h j e", h=HL, j=NCH)
            for j in range(NCH):
                ps = pspool.tile([128, HWID], f32, tag="ps")
                for d in range(DC):
                    nc.tensor.matmul(
                        ps[:], vT_in[:, d, j * 128:(j + 1) * 128], wv_in[:, d, :],
                        start=(d == 0), stop=(d == DC - 1),
                    )
                nc.vector.tensor_copy(
                    vaug_r[:, :, j, 0:64], ps[:].rearrange("p (h e) -> p h e", h=HL)
                )
            nc.vector.tensor_copy(
                vaug_r[:, :, :, 64],
                msk[:].rearrange("p (u j) -> p u j", u=1).broadcast_to([128, HL, NCH]),
            )

            # ---- attention per head: scores^T -> exp -> (v_aug^T @ exp) accum
            a0 = cpool.tile([128, NQ], f16)
            a1 = cpool.tile([64, NQ], f16)
            for h in range(HL):
                if h == 0:
                    kh, qh, adst = k0[0:64, :], q0[0:64, :], a0[0:64, :]
                elif h == 1:
                    kh, qh, adst = k0[64:128, :], q0[64:128, :], a0[64:128, :]
                else:
                    kh, qh, adst = k1[:, :], q1[:, :], a1[:, :]
                at = psapool.tile([65, NQ], f32)
                for j in range(NCH):
                    sc = pspool.tile([128, NQ], f32, tag="ps")
                    for nf in range(0, NQ, 512):
                        nc.tensor.matmul(
                            sc[:, nf:nf + 512], kh[:, j * 128:(j + 1) * 128],
                            qh[:, nf:nf + 512], start=True, stop=True,
                        )
                    ex = epool.tile([128, NQ], f16)
                    nc.scalar.activation(ex[:], sc[:], EXPF, scale=SCALE)
                    for nf in range(0, NQ, 512):
                        nc.tensor.matmul(
                            at[:, nf:nf + 512],
                            vaug[:, (h * NCH + j) * 65:(h * NCH + j) * 65 + 65],
                            ex[:, nf:nf + 512],
                            start=(j == 0), stop=(j == NCH - 1),
                        )
                rz = wpool.tile([1, NQ], f32)
                nc.vector.reciprocal(rz[:], at[64:65, :])
                rzb = wpool.tile([64, NQ], f32)
                nc.gpsimd.partition_broadcast(rzb[:], rz[:])
                nc.vector.tensor_mul(adst, at[0:64, :], rzb[:])

            # ---- output projection (row-split Wo): partial = attn_loc @ WoT
            for nt in range(NQ // 128):
                po = pspool.tile([128, D], f32, tag="ps")
                for kk, (asrc, kw) in enumerate(((a0, 128), (a1, 64))):
                    for nf in range(0, D, 512):
                        wf = min(512, D - nf)
                        nc.tensor.matmul(
                            po[:, nf:nf + wf],
                            asrc[:, nt * 128:(nt + 1) * 128],
                            wo_in[0:kw, kk, nf:nf + wf],
                            start=(kk == 0), stop=(kk == 1),
                        )
                ob = wpool.tile([128, D], f32, tag="ob")
                nc.vector.tensor_copy(ob[:], po[:])
                nc.sync.dma_start(out[nt * 128:(nt + 1) * 128, :], ob[:])

    nc.compile()
    return nc


def _get_program(SP: int):
    if SP not in _programs:
        _programs[SP] = _build(SP)
    return _programs[SP]


def kernel(query, key, value, mask, Wq, Wk, Wv, Wo, bo):
    query = np.asarray(query, np.float32)
    key = np.asarray(key, np.float32)
    value = np.asarray(value, np.float32)
    mask = np.asarray(mask, np.float32)
    Wq = np.asarray(Wq, np.float32)
    Wk = np.asarray(Wk, np.float32)
    Wv = np.asarray(Wv, np.float32)
    Wo = np.asarray(Wo, np.float32)
    bo = np.asarray(bo, np.float32)

    B, N, _ = query.shape
    idxs = [np.nonzero(mask[b] > 0.5)[0] for b in range(B)]
    se_max = max(len(i) for i in idxs)
    SP = max(((se_max + 127) // 128) * 128, 128)
    nc = _get_program(SP)

    in_maps = []
    for c in range(8):
        b, g = c // 4, c % 4
        hs = g * HWID
        idx = idxs[b]
        ne = len(idx)
        kTc = np.zeros((D, SP), np.float16)
        kTc[:, :ne] = key[b].T[:, idx].astype(np.float16)
        vTc = np.zeros((D, SP), np.float16)
        vTc[:, :ne] = value[b].T[:, idx].astype(np.float16)
        mvec = np.zeros((SP,), np.float16)
        mvec[:ne] = 1.0
        in_maps.append({
            "qT": np.ascontiguousarray(query[b].T.astype(np.float16)),
            "kT": kTc,
            "vT": vTc,
            "mv": mvec,
            "wqT": np.ascontiguousarray(Wq[hs:hs + HWID, :].T.astype(np.float16)),
            "wkT": np.ascontiguousarray(Wk[hs:hs + HWID, :].T.astype(np.float16)),
            "wvT": np.ascontiguousarray(Wv[hs:hs + HWID, :].T.astype(np.float16)),
            "woT": np.ascontiguousarray(Wo[:, hs:hs + HWID].T.astype(np.float16)),
        })

    res = run_bass_kernel_spmd(nc, in_maps, list(range(8))).results
    out = np.zeros((B, N, D), np.float32)
    for b in range(B):
        out[b] = res[4 * b]["out"] + res[4 * b + 1]["out"] \
            + res[4 * b + 2]["out"] + res[4 * b + 3]["out"] + bo
    return out
